# revision 6
# baseline (speedup 1.0000x reference)
"""Trainium2 Bass kernel for Attn_PointLevel (sparse_attention).

Math (per (b,v,p) patch, L=48 tokens, D=512):
  q = Xq @ Wq.T + bq ; k = Xk @ Wkv.T + bkv ; v = Xv @ Wkv.T + bkv
  S = q @ k.T  (48x48), diagonal masked to -inf
  A = softmax(S / sqrt(D)) ;  O = A @ v ;  Y = O @ Wo.T + bo

Kernel strategy (8 cores data-parallel over B; per core T = 7*24*48 = 8064
tokens, all PE matmuls bf16 at 1 cyc/row):
  Host folding: M = Wkv.T @ Wq, WVO = Wo @ Wkv, c1 = Wkv.T @ bq,
  bob = bo + Wo @ bkv.  Only terms that matter survive: the l-only score
  terms cancel in softmax; v's bkv folds into bob because A rows sum to 1.

  Per 384-token chunk (feature-major pipeline):
    G  = M @ XqT + c1              (PE + Act copy w/ per-partition bias)
    ST = XkT-pair-stationary @ G   (96x96 per patch-pair, K=512)
    ET = exp(scale*ST) * mask01    (Act exp, DVE 0/1 mask; cross-patch and
                                    diagonal killed; normalization DEFERRED)
    Zc[tok,1] = ET-stationary @ ones  (transposed col sums for ~free:
                                       output free size 1 per 128-tok tile)
    U  = Xv-pair-stationary @ ET   (= (E @ Xv).T, feature-major)
    Y  = U.T @ WVO.T -> token-major PSUM
    ysb = (Y * 1/Zc) + bob         (one fused DVE scalar_tensor_tensor)

  DMA plan (the cost model serializes every DMA through one HWDGE issue
  port, one exclusive DMA-engine pool, and a 16-deep completion-semaphore
  rotation that chains DMA issue to the completion of the DMA ~16 slots
  earlier -- FEWER, BIGGER DMAs win):
    - Xq and Xk are host-interleaved per 1152-token super-chunk so ONE
      xbar-transpose DMA per 128-feature chunk moves both (4 DMAs/super
      on the SP HWDGE queue)
    - Xv: one DMA per super-chunk (Pool SWDGE; super 0 on Act HWDGE so it
      lands before super 1's loads at startup)
    - all constants packed into ONE [128, 4709] bf16 block (Act HWDGE)
    - Y stored bf16, one DMA per super-chunk (Pool SWDGE); final super
      split per chunk/tile to shorten the drain
  PSUM: psq double-buffered (PRJB=2) so G(ec+1) overlaps the Act copy of
  G(ec) -- single-buffering was a 1.4us/ec ping-pong stall.
"""

import numpy as np

B, V, P, L, D = 8, 7, 24, 48, 512
T = V * P * L            # 8064 tokens per core
NCORES = 8
PAIR = 2 * L             # 96 tokens (2 patches) per attention tile
CH = 384                 # tokens per pipeline chunk (3x128 = 4x96)
SUPH = 3 * CH            # super-chunk for host-side xq/xk interleaving
SCALE = float(D) ** -0.5

_CACHE = {}


def _build(tokens, XINB=3, XTB=3, QKB=2, ATB=2, PSSB=2, PSUB=1, PSYB=2,
           PSZB=1, OUTB=3, PRJB=2):
    import concourse.mybir as mybir
    import concourse.tile as tile
    from concourse import bacc

    f32 = mybir.dt.float32
    bf16 = mybir.dt.bfloat16
    AF = mybir.ActivationFunctionType

    nchunks = tokens // CH
    assert tokens % CH == 0

    nc = bacc.Bacc("TRN2", target_bir_lowering=False)

    # xq and xk interleaved per super-chunk: [s][q|k][SUP, D] — lets one
    # xbar-transpose DMA per d-chunk move both tensors at once
    xqk_d = nc.dram_tensor("xqk", [2 * tokens, D], bf16, kind="ExternalInput")
    xv_d = nc.dram_tensor("xv", [tokens, D], bf16, kind="ExternalInput")
    CW = 4 * D + 4 * D + D + PAIR + 1 + 4    # packed const width (4709)
    cst_d = nc.dram_tensor("cst", [128, CW], bf16, kind="ExternalInput")
    y_d = nc.dram_tensor("y", [tokens, D], bf16, kind="ExternalOutput")

    NT = CH // 128           # 3 output tiles per chunk
    NP = CH // PAIR          # 4 pairs per chunk
    SUP = 3 * CH             # xv/ysb super-chunk (1152 tokens)
    TSUP = 7 * CH            # transpose super-chunk (2688 tokens)
    nsup = tokens // SUP
    assert tokens % SUP == 0 and tokens % TSUP == 0

    with tile.TileContext(nc) as tc:
        with (
            tc.tile_pool(name="const", bufs=1) as constp,
            tc.tile_pool(name="xin", bufs=XINB) as xinp,
            tc.tile_pool(name="xt", bufs=XTB) as xtp,
            tc.tile_pool(name="qkt", bufs=QKB) as qktp,
            tc.tile_pool(name="attn", bufs=ATB) as attnp,
            tc.tile_pool(name="outp", bufs=OUTB) as outp,
            tc.tile_pool(name="ps_proj", bufs=PRJB, space="PSUM") as ps_proj,
            tc.tile_pool(name="ps_s", bufs=PSSB, space="PSUM") as ps_s,
            tc.tile_pool(name="ps_u", bufs=PSUB, space="PSUM") as ps_u,
            tc.tile_pool(name="ps_zc", bufs=PSZB, space="PSUM") as ps_zc,
            tc.tile_pool(name="ps_y", bufs=PSYB, space="PSUM") as ps_y,
        ):
            # ALL constants in ONE DMA on the Act HWDGE queue: fewest links
            # in the sem-chain (emitted after the first transpose batch so
            # the xq transposes head the startup DMA chain)
            cst = constp.tile([128, CW], bf16, tag="cst")

            def emit_cst():
                nc.scalar.dma_start(cst, cst_d[:])

            CA = 4 * D + PAIR + 5    # critical prefix width

            def mt(dc, e0, e1):
                return cst[:, dc * D + e0 : dc * D + e1]

            mask01t = cst[:PAIR, 4 * D : 4 * D + PAIR]
            onesc = cst[:PAIR, 4 * D + PAIR : 4 * D + PAIR + 1]

            def c1(ec):
                return cst[:, 4 * D + PAIR + 1 + ec : 4 * D + PAIR + 2 + ec]

            def wvot(dc):
                return cst[:, CA + dc * D : CA + (dc + 1) * D]

            bob = cst[:, CA + 4 * D : CA + 5 * D]

            def issue_transposes(s):
                # xbar-transposed Xq+Xk (feature-major) for a whole
                # super-chunk (3 compute chunks) in 4 HWDGE DMAs on SP
                t0 = s * 2 * SUP
                xt = xtp.tile([128, 4, 2 * SUP], bf16, tag="xt")
                for dc in range(4):
                    nc.sync.dma_start_transpose(
                        xt[:, dc, :],
                        xqk_d[t0 : t0 + 2 * SUP, dc * 128 : (dc + 1) * 128],
                    )
                return xt

            def issue_xv(s, engine=None):
                # Xv pair tiles for a whole super-chunk in ONE SWDGE dma
                xv = xinp.tile([PAIR, 3 * NP, D], bf16, tag="xv")
                (engine or nc.gpsimd).dma_start(
                    xv,
                    xv_d[s * SUP : (s + 1) * SUP, :].rearrange(
                        "(j p) d -> p j d", p=PAIR
                    ),
                )
                return xv

            pend_t = [issue_transposes(0)]
            emit_cst()
            pend_v = [issue_xv(0, engine=nc.scalar)]

            # remaining constants (needed mid-chunk-0 or later)

            for c in range(nchunks):
                t0 = c * CH
                if c % 3 == 0:
                    xt_s = pend_t.pop(0)
                    xv_s = pend_v.pop(0)
                    ysb_s = outp.tile([128, 3 * NT, D], bf16, tag="ysb")
                if c % 3 == 1 and c + 2 < nchunks:
                    pend_t.append(issue_transposes(c // 3 + 1))
                if c % 3 == 2 and c + 1 < nchunks:
                    pend_v.append(issue_xv(c // 3 + 1))
                off = (c % 3) * CH
                xqt = xt_s[:, :, off : off + CH]
                xkt = xt_s[:, :, SUP + off : SUP + off + CH]

                # ---- G = M @ XqT + c1 (feature-major)
                gt = qktp.tile([128, 4, CH], bf16, tag="gt")
                for ec in range(4):
                    psq = ps_proj.tile([128, CH], f32, tag="proj")
                    for dc in range(4):
                        nc.tensor.matmul(
                            psq,
                            mt(dc, ec * 128, (ec + 1) * 128),
                            xqt[:, dc, :],
                            start=(dc == 0),
                            stop=(dc == 3),
                        )
                    nc.scalar.activation(
                        gt[:, ec, :], psq, AF.Identity,
                        bias=c1(ec),
                    )

                # ---- attention per pair: ST -> ET -> Zc, U
                pszc = ps_zc.tile([128, NT], f32, tag="zc")
                u = attnp.tile([128, 4, CH], bf16, tag="u")
                etw = attnp.tile([PAIR, CH], bf16, tag="etw")
                for j in range(NP):
                    ls = slice(j * PAIR, (j + 1) * PAIR)
                    pss = ps_s.tile([PAIR, PAIR], f32, tag="s")
                    for ec in range(4):
                        nc.tensor.matmul(
                            pss,
                            xkt[:, ec, ls],
                            gt[:, ec, ls],
                            start=(ec == 0),
                            stop=False,
                        )
                    nc.scalar.activation(etw[:, ls], pss, AF.Exp, scale=SCALE)
                    nc.vector.tensor_mul(etw[:, ls], etw[:, ls], mask01t)
                    # U[d, l] = sum_m Xv[m, d] * ET[m, l]
                    psu = ps_u.tile([128, 4, PAIR], f32, tag="u")
                    for dc in range(4):
                        nc.tensor.matmul(
                            psu[:, dc, :],
                            xv_s[:, (c % 3) * NP + j, dc * 128 : (dc + 1) * 128],
                            etw[:, ls],
                            start=True,
                            stop=True,
                        )
                    nc.vector.tensor_copy(u[:, :, ls], psu)

                # ---- Zc[token,1] per 128-token tile: ET-stationary @ ones
                # gives the transposed column sums directly (free size 1 on PE)
                for lt in range(NT):
                    nc.tensor.matmul(
                        pszc[:, lt : lt + 1],
                        etw[:, lt * 128 : (lt + 1) * 128],
                        onesc,
                        start=True,
                        stop=True,
                    )
                zcs = attnp.tile([128, NT], f32, tag="zcs")
                nc.vector.reciprocal(zcs, pszc)

                # ---- Y = (U.T @ WVO.T) * (1/Zc) + bob  (token-major, bf16)
                # fused scale+bias on DVE keeps Activation free for gt copies
                for lt in range(NT):
                    lsl = slice(lt * 128, (lt + 1) * 128)
                    psy = ps_y.tile([128, D], f32, tag="y")
                    for dc in range(4):
                        nc.tensor.matmul(
                            psy,
                            u[:, dc, lsl],
                            wvot(dc),
                            start=(dc == 0),
                            stop=(dc == 3),
                        )
                    nc.vector.scalar_tensor_tensor(
                        ysb_s[:, (c % 3) * NT + lt, :], psy, zcs[:, lt : lt + 1],
                        bob,
                        op0=mybir.AluOpType.mult, op1=mybir.AluOpType.add,
                    )
                last_super = c >= nchunks - 3
                if c == nchunks - 1:
                    # final chunk: store each tile as soon as ready, on the
                    # Act HWDGE queue (faster issue than Pool SWDGE gen)
                    for lt2 in range(NT):
                        nc.scalar.dma_start(
                            y_d[t0 + lt2 * 128 : t0 + (lt2 + 1) * 128, :].rearrange(
                                "(t p) e -> p t e", p=128
                            ),
                            ysb_s[:, (c % 3) * NT + lt2 : (c % 3) * NT + lt2 + 1, :],
                        )
                elif last_super:
                    # per-chunk stores in the final super shorten the drain
                    nc.gpsimd.dma_start(
                        y_d[t0 : t0 + CH, :].rearrange("(t p) e -> p t e", p=128),
                        ysb_s[:, (c % 3) * NT : (c % 3 + 1) * NT, :],
                    )
                elif c % 3 == 2:
                    # one SWDGE store per super-chunk on Pool
                    nc.gpsimd.dma_start(
                        y_d[t0 + CH - SUP : t0 + CH, :].rearrange(
                            "(t p) e -> p t e", p=128
                        ),
                        ysb_s,
                    )

    nc.compile()
    return nc


def _host_inputs(queries, keys, values, Wq, bq, Wkv, bkv, Wo, bo, tokens):
    import ml_dtypes

    bf16 = ml_dtypes.bfloat16
    M = Wkv.astype(np.float64).T @ Wq.astype(np.float64)
    WVO = Wo.astype(np.float64) @ Wkv.astype(np.float64)
    # packed const block [128, 4D+4D+D+PAIR+1+4] (see _build)
    CW = 9 * D + PAIR + 5
    CA = 4 * D + PAIR + 5
    cst = np.zeros((128, CW), np.float32)
    cst[:, 0 : 4 * D] = M.T.reshape(4, 128, D).transpose(1, 0, 2).reshape(128, 4 * D)
    m01 = np.ones((PAIR, PAIR), np.float32)
    m01[:L, :L] -= np.eye(L)
    m01[L:, L:] -= np.eye(L)
    m01[:L, L:] = 0.0
    m01[L:, :L] = 0.0
    cst[:PAIR, 4 * D : 4 * D + PAIR] = m01
    cst[:, 4 * D + PAIR] = 1.0
    c1v = Wkv.astype(np.float64).T @ bq.astype(np.float64)
    cst[:, 4 * D + PAIR + 1 : CA] = c1v.reshape(4, 128).T
    cst[:, CA : CA + 4 * D] = (
        WVO.T.reshape(4, 128, D).transpose(1, 0, 2).reshape(128, 4 * D)
    )
    bo_eff = (bo.astype(np.float64) + Wo.astype(np.float64) @ bkv.astype(np.float64))
    cst[:, CA + 4 * D :] = bo_eff[None, :]
    shared = dict(cst=cst.astype(bf16))
    in_maps = []
    for core in range(NCORES):
        m = dict(shared)
        xq = queries[core].reshape(-1, D)[:tokens]
        xk = keys[core].reshape(-1, D)[:tokens]
        nsup = tokens // SUPH
        xqk = np.stack(
            [xq.reshape(nsup, SUPH, D), xk.reshape(nsup, SUPH, D)], axis=1
        ).reshape(2 * tokens, D)
        m["xqk"] = xqk.astype(bf16)
        m["xv"] = values[core].reshape(-1, D)[:tokens].astype(bf16)
        in_maps.append(m)
    return in_maps


def kernel(queries, keys, values, Wq, bq, Wkv, bkv, Wo, bo, _tokens=T, _trace=False):
    queries = np.asarray(queries)
    keys = np.asarray(keys)
    values = np.asarray(values)
    from concourse.bass_utils import run_bass_kernel_spmd

    key = _tokens
    if key not in _CACHE:
        _CACHE[key] = _build(_tokens)
    nc = _CACHE[key]

    in_maps = _host_inputs(
        queries, keys, values,
        np.asarray(Wq), np.asarray(bq), np.asarray(Wkv), np.asarray(bkv),
        np.asarray(Wo), np.asarray(bo), _tokens,
    )
    res = run_bass_kernel_spmd(
        nc, in_maps, core_ids=list(range(NCORES)), trace=_trace,
    )
    outs = [np.asarray(res.results[i]["y"]).astype(np.float32) for i in range(NCORES)]
    if _tokens == T:
        full = np.stack([o.reshape(V, P, L, D) for o in outs], axis=0)
    else:
        full = np.stack(outs, axis=0)
    if _trace:
        return full, res
    return full


# revision 7
# speedup vs baseline: 1.0028x; 1.0028x over previous
"""Trainium2 Bass kernel for Attn_PointLevel (sparse_attention).

Math (per (b,v,p) patch, L=48 tokens, D=512):
  q = Xq @ Wq.T + bq ; k = Xk @ Wkv.T + bkv ; v = Xv @ Wkv.T + bkv
  S = q @ k.T  (48x48), diagonal masked to -inf
  A = softmax(S / sqrt(D)) ;  O = A @ v ;  Y = O @ Wo.T + bo

Kernel strategy (8 cores data-parallel over B; per core T = 7*24*48 = 8064
tokens, all PE matmuls bf16 at 1 cyc/row):
  Host folding: M = Wkv.T @ Wq, WVO = Wo @ Wkv, c1 = Wkv.T @ bq,
  bob = bo + Wo @ bkv.  Only terms that matter survive: the l-only score
  terms cancel in softmax; v's bkv folds into bob because A rows sum to 1.

  Per 384-token chunk (feature-major pipeline):
    G  = M @ XqT + c1              (PE + Act copy w/ per-partition bias)
    ST = XkT-pair-stationary @ G   (96x96 per patch-pair, K=512)
    ET = exp(scale*ST) * mask01    (Act exp, DVE 0/1 mask; cross-patch and
                                    diagonal killed; normalization DEFERRED)
    Zc[tok,1] = ET-stationary @ ones  (transposed col sums for ~free:
                                       output free size 1 per 128-tok tile)
    U  = Xv-pair-stationary @ ET   (= (E @ Xv).T, feature-major)
    Y  = U.T @ WVO.T -> token-major PSUM
    ysb = (Y * 1/Zc) + bob         (one fused DVE scalar_tensor_tensor)

  DMA plan (the cost model serializes every DMA through one HWDGE issue
  port, one exclusive DMA-engine pool, and a 16-deep completion-semaphore
  rotation that chains DMA issue to the completion of the DMA ~16 slots
  earlier -- FEWER, BIGGER DMAs win):
    - Xq and Xk are host-interleaved per 1152-token super-chunk so ONE
      xbar-transpose DMA per 128-feature chunk moves both (4 DMAs/super
      on the SP HWDGE queue)
    - Xv: one DMA per super-chunk (Pool SWDGE; super 0 on Act HWDGE so it
      lands before super 1's loads at startup)
    - all constants packed into ONE [128, 4709] bf16 block (Act HWDGE)
    - Y stored bf16, one DMA per super-chunk (Pool SWDGE); final super
      split per chunk/tile to shorten the drain
  PSUM: psq double-buffered (PRJB=2) so G(ec+1) overlaps the Act copy of
  G(ec) -- single-buffering was a 1.4us/ec ping-pong stall.
"""

import numpy as np

B, V, P, L, D = 8, 7, 24, 48, 512
T = V * P * L            # 8064 tokens per core
NCORES = 8
PAIR = 2 * L             # 96 tokens (2 patches) per attention tile
CH = 384                 # tokens per pipeline chunk (3x128 = 4x96)
SUPH = 3 * CH            # super-chunk for host-side xq/xk interleaving
SCALE = float(D) ** -0.5

_CACHE = {}


def _build(tokens, XINB=3, XTB=3, QKB=2, ATB=2, PSSB=2, PSUB=1, PSYB=2,
           PSZB=1, OUTB=3, PRJB=2):
    import concourse.mybir as mybir
    import concourse.tile as tile
    from concourse import bacc

    f32 = mybir.dt.float32
    bf16 = mybir.dt.bfloat16
    AF = mybir.ActivationFunctionType

    nchunks = tokens // CH
    assert tokens % CH == 0

    nc = bacc.Bacc("TRN2", target_bir_lowering=False)

    # xq and xk interleaved per super-chunk: [s][q|k][SUP, D] — lets one
    # xbar-transpose DMA per d-chunk move both tensors at once
    xqk_d = nc.dram_tensor("xqk", [2 * tokens, D], bf16, kind="ExternalInput")
    xv_d = nc.dram_tensor("xv", [tokens, D], bf16, kind="ExternalInput")
    CW = 4 * D + 4 * D + D + PAIR + 1 + 4    # packed const width (4709)
    cst_d = nc.dram_tensor("cst", [128, CW], bf16, kind="ExternalInput")
    y_d = nc.dram_tensor("y", [tokens, D], bf16, kind="ExternalOutput")

    NT = CH // 128           # 3 output tiles per chunk
    NP = CH // PAIR          # 4 pairs per chunk
    SUP = 3 * CH             # xv/ysb super-chunk (1152 tokens)
    TSUP = 7 * CH            # transpose super-chunk (2688 tokens)
    nsup = tokens // SUP
    assert tokens % SUP == 0 and tokens % TSUP == 0

    with tile.TileContext(nc) as tc:
        with (
            tc.tile_pool(name="const", bufs=1) as constp,
            tc.tile_pool(name="xin", bufs=XINB) as xinp,
            tc.tile_pool(name="xt", bufs=XTB) as xtp,
            tc.tile_pool(name="qkt", bufs=QKB) as qktp,
            tc.tile_pool(name="attn", bufs=ATB) as attnp,
            tc.tile_pool(name="outp", bufs=OUTB) as outp,
            tc.tile_pool(name="ps_proj", bufs=PRJB, space="PSUM") as ps_proj,
            tc.tile_pool(name="ps_s", bufs=PSSB, space="PSUM") as ps_s,
            tc.tile_pool(name="ps_u", bufs=PSUB, space="PSUM") as ps_u,
            tc.tile_pool(name="ps_zc", bufs=PSZB, space="PSUM") as ps_zc,
            tc.tile_pool(name="ps_y", bufs=PSYB, space="PSUM") as ps_y,
        ):
            # ALL constants in ONE DMA on the Act HWDGE queue: fewest links
            # in the sem-chain (emitted after the first transpose batch so
            # the xq transposes head the startup DMA chain)
            cst = constp.tile([128, CW], bf16, tag="cst")

            def emit_cst():
                nc.scalar.dma_start(cst, cst_d[:])

            CA = 4 * D + PAIR + 5    # critical prefix width

            def mt(dc, e0, e1):
                return cst[:, dc * D + e0 : dc * D + e1]

            mask01t = cst[:PAIR, 4 * D : 4 * D + PAIR]
            onesc = cst[:PAIR, 4 * D + PAIR : 4 * D + PAIR + 1]

            def c1(ec):
                return cst[:, 4 * D + PAIR + 1 + ec : 4 * D + PAIR + 2 + ec]

            def wvot(dc):
                return cst[:, CA + dc * D : CA + (dc + 1) * D]

            bob = cst[:, CA + 4 * D : CA + 5 * D]

            def issue_transposes(s):
                # xbar-transposed Xq+Xk (feature-major) for a whole
                # super-chunk (3 compute chunks) in 4 HWDGE DMAs on SP
                t0 = s * 2 * SUP
                xt = xtp.tile([128, 4, 2 * SUP], bf16, tag="xt")
                for dc in range(4):
                    nc.sync.dma_start_transpose(
                        xt[:, dc, :],
                        xqk_d[t0 : t0 + 2 * SUP, dc * 128 : (dc + 1) * 128],
                    )
                return xt

            def issue_xv(s, engine=None):
                # Xv pair tiles for a whole super-chunk in ONE SWDGE dma
                xv = xinp.tile([PAIR, 3 * NP, D], bf16, tag="xv")
                (engine or nc.gpsimd).dma_start(
                    xv,
                    xv_d[s * SUP : (s + 1) * SUP, :].rearrange(
                        "(j p) d -> p j d", p=PAIR
                    ),
                )
                return xv

            pend_t = [issue_transposes(0)]
            emit_cst()
            pend_v = [issue_xv(0, engine=nc.scalar)]

            # remaining constants (needed mid-chunk-0 or later)

            for c in range(nchunks):
                t0 = c * CH
                if c % 3 == 0:
                    xt_s = pend_t.pop(0)
                    xv_s = pend_v.pop(0)
                    ysb_s = outp.tile([128, 3 * NT, D], bf16, tag="ysb")
                if c % 3 == 1 and c + 2 < nchunks:
                    pend_t.append(issue_transposes(c // 3 + 1))
                if c % 3 == 2 and c + 1 < nchunks:
                    pend_v.append(issue_xv(c // 3 + 1))
                off = (c % 3) * CH
                xqt = xt_s[:, :, off : off + CH]
                xkt = xt_s[:, :, SUP + off : SUP + off + CH]

                # ---- G = M @ XqT + c1 (feature-major)
                gt = qktp.tile([128, 4, CH], bf16, tag="gt")
                for ec in range(4):
                    psq = ps_proj.tile([128, CH], f32, tag="proj")
                    for dc in range(4):
                        nc.tensor.matmul(
                            psq,
                            mt(dc, ec * 128, (ec + 1) * 128),
                            xqt[:, dc, :],
                            start=(dc == 0),
                            stop=(dc == 3),
                        )
                    nc.scalar.activation(
                        gt[:, ec, :], psq, AF.Identity,
                        bias=c1(ec),
                    )

                # ---- attention per pair: ST -> ET -> Zc, U
                pszc = ps_zc.tile([128, NT], f32, tag="zc")
                u = attnp.tile([128, 4, CH], bf16, tag="u")
                etw = attnp.tile([PAIR, CH], bf16, tag="etw")
                for j in range(NP):
                    ls = slice(j * PAIR, (j + 1) * PAIR)
                    pss = ps_s.tile([PAIR, PAIR], f32, tag="s")
                    for ec in range(4):
                        nc.tensor.matmul(
                            pss,
                            xkt[:, ec, ls],
                            gt[:, ec, ls],
                            start=(ec == 0),
                            stop=False,
                        )
                    nc.scalar.activation(etw[:, ls], pss, AF.Exp, scale=SCALE)
                    nc.vector.tensor_mul(etw[:, ls], etw[:, ls], mask01t)
                    # U[d, l] = sum_m Xv[m, d] * ET[m, l]
                    psu = ps_u.tile([128, 4, PAIR], f32, tag="u")
                    for dc in range(4):
                        nc.tensor.matmul(
                            psu[:, dc, :],
                            xv_s[:, (c % 3) * NP + j, dc * 128 : (dc + 1) * 128],
                            etw[:, ls],
                            start=True,
                            stop=True,
                        )
                    if j % 2 == 0:
                        nc.vector.tensor_copy(u[:, :, ls], psu)
                    else:
                        nc.scalar.activation(u[:, :, ls], psu, AF.Identity)

                # ---- Zc[token,1] per 128-token tile: ET-stationary @ ones
                # gives the transposed column sums directly (free size 1 on PE)
                for lt in range(NT):
                    nc.tensor.matmul(
                        pszc[:, lt : lt + 1],
                        etw[:, lt * 128 : (lt + 1) * 128],
                        onesc,
                        start=True,
                        stop=True,
                    )
                zcs = attnp.tile([128, NT], f32, tag="zcs")
                nc.vector.reciprocal(zcs, pszc)

                # ---- Y = (U.T @ WVO.T) * (1/Zc) + bob  (token-major, bf16)
                # fused scale+bias on DVE keeps Activation free for gt copies
                for lt in range(NT):
                    lsl = slice(lt * 128, (lt + 1) * 128)
                    psy = ps_y.tile([128, D], f32, tag="y")
                    for dc in range(4):
                        nc.tensor.matmul(
                            psy,
                            u[:, dc, lsl],
                            wvot(dc),
                            start=(dc == 0),
                            stop=(dc == 3),
                        )
                    nc.vector.scalar_tensor_tensor(
                        ysb_s[:, (c % 3) * NT + lt, :], psy, zcs[:, lt : lt + 1],
                        bob,
                        op0=mybir.AluOpType.mult, op1=mybir.AluOpType.add,
                    )
                last_super = c >= nchunks - 3
                if c == nchunks - 1:
                    # final chunk: store each tile as soon as ready, on the
                    # Act HWDGE queue (faster issue than Pool SWDGE gen)
                    for lt2 in range(NT):
                        nc.scalar.dma_start(
                            y_d[t0 + lt2 * 128 : t0 + (lt2 + 1) * 128, :].rearrange(
                                "(t p) e -> p t e", p=128
                            ),
                            ysb_s[:, (c % 3) * NT + lt2 : (c % 3) * NT + lt2 + 1, :],
                        )
                elif last_super:
                    # per-chunk stores in the final super shorten the drain
                    nc.gpsimd.dma_start(
                        y_d[t0 : t0 + CH, :].rearrange("(t p) e -> p t e", p=128),
                        ysb_s[:, (c % 3) * NT : (c % 3 + 1) * NT, :],
                    )
                elif c % 3 == 2:
                    # one SWDGE store per super-chunk on Pool
                    nc.gpsimd.dma_start(
                        y_d[t0 + CH - SUP : t0 + CH, :].rearrange(
                            "(t p) e -> p t e", p=128
                        ),
                        ysb_s,
                    )

    nc.compile()
    return nc


def _host_inputs(queries, keys, values, Wq, bq, Wkv, bkv, Wo, bo, tokens):
    import ml_dtypes

    bf16 = ml_dtypes.bfloat16
    M = Wkv.astype(np.float64).T @ Wq.astype(np.float64)
    WVO = Wo.astype(np.float64) @ Wkv.astype(np.float64)
    # packed const block [128, 4D+4D+D+PAIR+1+4] (see _build)
    CW = 9 * D + PAIR + 5
    CA = 4 * D + PAIR + 5
    cst = np.zeros((128, CW), np.float32)
    cst[:, 0 : 4 * D] = M.T.reshape(4, 128, D).transpose(1, 0, 2).reshape(128, 4 * D)
    m01 = np.ones((PAIR, PAIR), np.float32)
    m01[:L, :L] -= np.eye(L)
    m01[L:, L:] -= np.eye(L)
    m01[:L, L:] = 0.0
    m01[L:, :L] = 0.0
    cst[:PAIR, 4 * D : 4 * D + PAIR] = m01
    cst[:, 4 * D + PAIR] = 1.0
    c1v = Wkv.astype(np.float64).T @ bq.astype(np.float64)
    cst[:, 4 * D + PAIR + 1 : CA] = c1v.reshape(4, 128).T
    cst[:, CA : CA + 4 * D] = (
        WVO.T.reshape(4, 128, D).transpose(1, 0, 2).reshape(128, 4 * D)
    )
    bo_eff = (bo.astype(np.float64) + Wo.astype(np.float64) @ bkv.astype(np.float64))
    cst[:, CA + 4 * D :] = bo_eff[None, :]
    shared = dict(cst=cst.astype(bf16))
    in_maps = []
    for core in range(NCORES):
        m = dict(shared)
        xq = queries[core].reshape(-1, D)[:tokens]
        xk = keys[core].reshape(-1, D)[:tokens]
        nsup = tokens // SUPH
        xqk = np.stack(
            [xq.reshape(nsup, SUPH, D), xk.reshape(nsup, SUPH, D)], axis=1
        ).reshape(2 * tokens, D)
        m["xqk"] = xqk.astype(bf16)
        m["xv"] = values[core].reshape(-1, D)[:tokens].astype(bf16)
        in_maps.append(m)
    return in_maps


def kernel(queries, keys, values, Wq, bq, Wkv, bkv, Wo, bo, _tokens=T, _trace=False):
    queries = np.asarray(queries)
    keys = np.asarray(keys)
    values = np.asarray(values)
    from concourse.bass_utils import run_bass_kernel_spmd

    key = _tokens
    if key not in _CACHE:
        _CACHE[key] = _build(_tokens)
    nc = _CACHE[key]

    in_maps = _host_inputs(
        queries, keys, values,
        np.asarray(Wq), np.asarray(bq), np.asarray(Wkv), np.asarray(bkv),
        np.asarray(Wo), np.asarray(bo), _tokens,
    )
    res = run_bass_kernel_spmd(
        nc, in_maps, core_ids=list(range(NCORES)), trace=_trace,
    )
    outs = [np.asarray(res.results[i]["y"]).astype(np.float32) for i in range(NCORES)]
    if _tokens == T:
        full = np.stack([o.reshape(V, P, L, D) for o in outs], axis=0)
    else:
        full = np.stack(outs, axis=0)
    if _trace:
        return full, res
    return full


# revision 8
# speedup vs baseline: 1.0042x; 1.0014x over previous
"""Trainium2 Bass kernel for Attn_PointLevel (sparse_attention).

Math (per (b,v,p) patch, L=48 tokens, D=512):
  q = Xq @ Wq.T + bq ; k = Xk @ Wkv.T + bkv ; v = Xv @ Wkv.T + bkv
  S = q @ k.T  (48x48), diagonal masked to -inf
  A = softmax(S / sqrt(D)) ;  O = A @ v ;  Y = O @ Wo.T + bo

Kernel strategy (8 cores data-parallel over B; per core T = 7*24*48 = 8064
tokens, all PE matmuls bf16 at 1 cyc/row):
  Host folding: M = Wkv.T @ Wq, WVO = Wo @ Wkv, c1 = Wkv.T @ bq,
  bob = bo + Wo @ bkv.  Only terms that matter survive: the l-only score
  terms cancel in softmax; v's bkv folds into bob because A rows sum to 1.

  Per 384-token chunk (feature-major pipeline):
    G  = M @ XqT + c1              (PE + Act copy w/ per-partition bias)
    ST = XkT-pair-stationary @ G   (96x96 per patch-pair, K=512)
    ET = exp(scale*ST) * mask01    (Act exp, DVE 0/1 mask; cross-patch and
                                    diagonal killed; normalization DEFERRED)
    Zc[tok,1] = ET-stationary @ ones  (transposed col sums for ~free:
                                       output free size 1 per 128-tok tile)
    U  = Xv-pair-stationary @ ET   (= (E @ Xv).T, feature-major)
    Y  = U.T @ WVO.T -> token-major PSUM
    ysb = (Y * 1/Zc) + bob         (one fused DVE scalar_tensor_tensor)

  DMA plan (the cost model serializes every DMA through one HWDGE issue
  port, one exclusive DMA-engine pool, and a 16-deep completion-semaphore
  rotation that chains DMA issue to the completion of the DMA ~16 slots
  earlier -- FEWER, BIGGER DMAs win):
    - Xq and Xk are host-interleaved per 1152-token super-chunk so ONE
      xbar-transpose DMA per 128-feature chunk moves both (4 DMAs/super
      on the SP HWDGE queue)
    - Xv: one DMA per super-chunk (Pool SWDGE; super 0 on Act HWDGE so it
      lands before super 1's loads at startup)
    - all constants packed into ONE [128, 4709] bf16 block (Act HWDGE)
    - Y stored bf16, one DMA per super-chunk (Pool SWDGE); final super
      split per chunk/tile to shorten the drain
  PSUM: psq double-buffered (PRJB=2) so G(ec+1) overlaps the Act copy of
  G(ec) -- single-buffering was a 1.4us/ec ping-pong stall.
"""

import numpy as np

B, V, P, L, D = 8, 7, 24, 48, 512
T = V * P * L            # 8064 tokens per core
NCORES = 8
PAIR = 2 * L             # 96 tokens (2 patches) per attention tile
CH = 384                 # tokens per pipeline chunk (3x128 = 4x96)
SUPH = 3 * CH            # super-chunk for host-side xq/xk interleaving
SCALE = float(D) ** -0.5

_CACHE = {}


def _build(tokens, XINB=3, XTB=3, QKB=2, ATB=2, PSSB=2, PSUB=1, PSYB=2,
           PSZB=1, OUTB=3, PRJB=2):
    import concourse.mybir as mybir
    import concourse.tile as tile
    from concourse import bacc

    f32 = mybir.dt.float32
    bf16 = mybir.dt.bfloat16
    AF = mybir.ActivationFunctionType

    nchunks = tokens // CH
    assert tokens % CH == 0

    nc = bacc.Bacc("TRN2", target_bir_lowering=False)

    # xq and xk interleaved per super-chunk: [s][q|k][SUP, D] — lets one
    # xbar-transpose DMA per d-chunk move both tensors at once
    xqk_d = nc.dram_tensor("xqk", [2 * tokens, D], bf16, kind="ExternalInput")
    xv_d = nc.dram_tensor("xv", [tokens, D], bf16, kind="ExternalInput")
    CW = 4 * D + 4 * D + D + PAIR + 1 + 4    # packed const width (4709)
    cst_d = nc.dram_tensor("cst", [128, CW], bf16, kind="ExternalInput")
    y_d = nc.dram_tensor("y", [tokens, D], bf16, kind="ExternalOutput")

    NT = CH // 128           # 3 output tiles per chunk
    NP = CH // PAIR          # 4 pairs per chunk
    SUP = 3 * CH             # xv/ysb super-chunk (1152 tokens)
    TSUP = 7 * CH            # transpose super-chunk (2688 tokens)
    nsup = tokens // SUP
    assert tokens % SUP == 0 and tokens % TSUP == 0

    with tile.TileContext(nc) as tc:
        with (
            tc.tile_pool(name="const", bufs=1) as constp,
            tc.tile_pool(name="xin", bufs=XINB) as xinp,
            tc.tile_pool(name="xt", bufs=XTB) as xtp,
            tc.tile_pool(name="qkt", bufs=QKB) as qktp,
            tc.tile_pool(name="attn", bufs=ATB) as attnp,
            tc.tile_pool(name="outp", bufs=OUTB) as outp,
            tc.tile_pool(name="ps_proj", bufs=PRJB, space="PSUM") as ps_proj,
            tc.tile_pool(name="ps_s", bufs=PSSB, space="PSUM") as ps_s,
            tc.tile_pool(name="ps_u", bufs=PSUB, space="PSUM") as ps_u,
            tc.tile_pool(name="ps_zc", bufs=PSZB, space="PSUM") as ps_zc,
            tc.tile_pool(name="ps_y", bufs=PSYB, space="PSUM") as ps_y,
        ):
            # ALL constants in ONE DMA on the Act HWDGE queue: fewest links
            # in the sem-chain (emitted after the first transpose batch so
            # the xq transposes head the startup DMA chain)
            cst = constp.tile([128, CW], bf16, tag="cst")

            def emit_cst():
                nc.scalar.dma_start(cst, cst_d[:])

            CA = 4 * D + PAIR + 5    # critical prefix width

            def mt(dc, e0, e1):
                return cst[:, dc * D + e0 : dc * D + e1]

            mask01t = cst[:PAIR, 4 * D : 4 * D + PAIR]
            onesc = cst[:PAIR, 4 * D + PAIR : 4 * D + PAIR + 1]

            def c1(ec):
                return cst[:, 4 * D + PAIR + 1 + ec : 4 * D + PAIR + 2 + ec]

            def wvot(dc):
                return cst[:, CA + dc * D : CA + (dc + 1) * D]

            bob = cst[:, CA + 4 * D : CA + 5 * D]

            def issue_transposes(s):
                # xbar-transposed Xq+Xk (feature-major) for a whole
                # super-chunk (3 compute chunks) in 4 HWDGE DMAs on SP
                t0 = s * 2 * SUP
                xt = xtp.tile([128, 4, 2 * SUP], bf16, tag="xt")
                for dc in range(4):
                    nc.sync.dma_start_transpose(
                        xt[:, dc, :],
                        xqk_d[t0 : t0 + 2 * SUP, dc * 128 : (dc + 1) * 128],
                    )
                return xt

            def issue_xv(s, engine=None):
                # Xv pair tiles for a whole super-chunk in ONE SWDGE dma
                xv = xinp.tile([PAIR, 3 * NP, D], bf16, tag="xv")
                (engine or nc.gpsimd).dma_start(
                    xv,
                    xv_d[s * SUP : (s + 1) * SUP, :].rearrange(
                        "(j p) d -> p j d", p=PAIR
                    ),
                )
                return xv

            pend_t = [issue_transposes(0)]
            emit_cst()
            pend_v = [issue_xv(0, engine=nc.scalar)]

            # remaining constants (needed mid-chunk-0 or later)

            for c in range(nchunks):
                t0 = c * CH
                if c % 3 == 0:
                    xt_s = pend_t.pop(0)
                    xv_s = pend_v.pop(0)
                    ysb_s = outp.tile([128, 3 * NT, D], bf16, tag="ysb")
                if c % 3 == 1 and c + 2 < nchunks:
                    pend_t.append(issue_transposes(c // 3 + 1))
                if c % 3 == 2 and c + 1 < nchunks:
                    pend_v.append(issue_xv(c // 3 + 1))
                off = (c % 3) * CH
                xqt = xt_s[:, :, off : off + CH]
                xkt = xt_s[:, :, SUP + off : SUP + off + CH]

                # ---- G = M @ XqT + c1 (feature-major)
                gt = qktp.tile([128, 4, CH], bf16, tag="gt")
                for ec in range(4):
                    psq = ps_proj.tile([128, CH], f32, tag="proj")
                    for dc in range(4):
                        nc.tensor.matmul(
                            psq,
                            mt(dc, ec * 128, (ec + 1) * 128),
                            xqt[:, dc, :],
                            start=(dc == 0),
                            stop=(dc == 3),
                        )
                    nc.scalar.activation(
                        gt[:, ec, :], psq, AF.Identity,
                        bias=c1(ec),
                    )

                # ---- attention per pair: ST -> ET -> Zc, U
                pszc = ps_zc.tile([128, NT], f32, tag="zc")
                u = attnp.tile([128, 4, CH], bf16, tag="u")
                etw = attnp.tile([PAIR, CH], bf16, tag="etw")
                for j in range(NP):
                    ls = slice(j * PAIR, (j + 1) * PAIR)
                    pss = ps_s.tile([PAIR, PAIR], f32, tag="s")
                    for ec in range(4):
                        nc.tensor.matmul(
                            pss,
                            xkt[:, ec, ls],
                            gt[:, ec, ls],
                            start=(ec == 0),
                            stop=False,
                        )
                    nc.scalar.activation(etw[:, ls], pss, AF.Exp, scale=SCALE)
                    nc.vector.tensor_mul(etw[:, ls], etw[:, ls], mask01t)
                    # U[d, l] = sum_m Xv[m, d] * ET[m, l]
                    psu = ps_u.tile([128, 4, PAIR], f32, tag="u")
                    for dc in range(4):
                        nc.tensor.matmul(
                            psu[:, dc, :],
                            xv_s[:, (c % 3) * NP + j, dc * 128 : (dc + 1) * 128],
                            etw[:, ls],
                            start=True,
                            stop=True,
                        )
                    if j % 2 == 0:
                        nc.vector.tensor_copy(u[:, :, ls], psu)
                    else:
                        nc.scalar.activation(u[:, :, ls], psu, AF.Identity)

                # ---- Zc[token,1] per 128-token tile: ET-stationary @ ones
                # gives the transposed column sums directly (free size 1 on PE)
                for lt in range(NT):
                    nc.tensor.matmul(
                        pszc[:, lt : lt + 1],
                        etw[:, lt * 128 : (lt + 1) * 128],
                        onesc,
                        start=True,
                        stop=True,
                    )
                zcs = attnp.tile([128, NT], f32, tag="zcs")
                nc.vector.reciprocal(zcs, pszc)

                # ---- Y = (U.T @ WVO.T) * (1/Zc) + bob  (token-major, bf16)
                # fused scale+bias on DVE keeps Activation free for gt copies
                for lt in range(NT):
                    lsl = slice(lt * 128, (lt + 1) * 128)
                    psy = ps_y.tile([128, D], f32, tag="y")
                    for dc in range(4):
                        nc.tensor.matmul(
                            psy,
                            u[:, dc, lsl],
                            wvot(dc),
                            start=(dc == 0),
                            stop=(dc == 3),
                        )
                    nc.vector.scalar_tensor_tensor(
                        ysb_s[:, (c % 3) * NT + lt, :], psy, zcs[:, lt : lt + 1],
                        bob,
                        op0=mybir.AluOpType.mult, op1=mybir.AluOpType.add,
                    )
                last_super = c >= nchunks - 3
                if c == nchunks - 1:
                    # final chunk: store each tile as soon as ready, on the
                    # Act HWDGE queue (faster issue than Pool SWDGE gen)
                    for lt2 in range(NT):
                        nc.sync.dma_start(
                            y_d[t0 + lt2 * 128 : t0 + (lt2 + 1) * 128, :].rearrange(
                                "(t p) e -> p t e", p=128
                            ),
                            ysb_s[:, (c % 3) * NT + lt2 : (c % 3) * NT + lt2 + 1, :],
                        )
                elif last_super:
                    # per-chunk stores in the final super shorten the drain
                    nc.sync.dma_start(
                        y_d[t0 : t0 + CH, :].rearrange("(t p) e -> p t e", p=128),
                        ysb_s[:, (c % 3) * NT : (c % 3 + 1) * NT, :],
                    )
                elif c % 3 == 2:
                    # one SWDGE store per super-chunk on Pool
                    nc.gpsimd.dma_start(
                        y_d[t0 + CH - SUP : t0 + CH, :].rearrange(
                            "(t p) e -> p t e", p=128
                        ),
                        ysb_s,
                    )

    nc.compile()
    return nc


def _host_inputs(queries, keys, values, Wq, bq, Wkv, bkv, Wo, bo, tokens):
    import ml_dtypes

    bf16 = ml_dtypes.bfloat16
    M = Wkv.astype(np.float64).T @ Wq.astype(np.float64)
    WVO = Wo.astype(np.float64) @ Wkv.astype(np.float64)
    # packed const block [128, 4D+4D+D+PAIR+1+4] (see _build)
    CW = 9 * D + PAIR + 5
    CA = 4 * D + PAIR + 5
    cst = np.zeros((128, CW), np.float32)
    cst[:, 0 : 4 * D] = M.T.reshape(4, 128, D).transpose(1, 0, 2).reshape(128, 4 * D)
    m01 = np.ones((PAIR, PAIR), np.float32)
    m01[:L, :L] -= np.eye(L)
    m01[L:, L:] -= np.eye(L)
    m01[:L, L:] = 0.0
    m01[L:, :L] = 0.0
    cst[:PAIR, 4 * D : 4 * D + PAIR] = m01
    cst[:, 4 * D + PAIR] = 1.0
    c1v = Wkv.astype(np.float64).T @ bq.astype(np.float64)
    cst[:, 4 * D + PAIR + 1 : CA] = c1v.reshape(4, 128).T
    cst[:, CA : CA + 4 * D] = (
        WVO.T.reshape(4, 128, D).transpose(1, 0, 2).reshape(128, 4 * D)
    )
    bo_eff = (bo.astype(np.float64) + Wo.astype(np.float64) @ bkv.astype(np.float64))
    cst[:, CA + 4 * D :] = bo_eff[None, :]
    shared = dict(cst=cst.astype(bf16))
    in_maps = []
    for core in range(NCORES):
        m = dict(shared)
        xq = queries[core].reshape(-1, D)[:tokens]
        xk = keys[core].reshape(-1, D)[:tokens]
        nsup = tokens // SUPH
        xqk = np.stack(
            [xq.reshape(nsup, SUPH, D), xk.reshape(nsup, SUPH, D)], axis=1
        ).reshape(2 * tokens, D)
        m["xqk"] = xqk.astype(bf16)
        m["xv"] = values[core].reshape(-1, D)[:tokens].astype(bf16)
        in_maps.append(m)
    return in_maps


def kernel(queries, keys, values, Wq, bq, Wkv, bkv, Wo, bo, _tokens=T, _trace=False):
    queries = np.asarray(queries)
    keys = np.asarray(keys)
    values = np.asarray(values)
    from concourse.bass_utils import run_bass_kernel_spmd

    key = _tokens
    if key not in _CACHE:
        _CACHE[key] = _build(_tokens)
    nc = _CACHE[key]

    in_maps = _host_inputs(
        queries, keys, values,
        np.asarray(Wq), np.asarray(bq), np.asarray(Wkv), np.asarray(bkv),
        np.asarray(Wo), np.asarray(bo), _tokens,
    )
    res = run_bass_kernel_spmd(
        nc, in_maps, core_ids=list(range(NCORES)), trace=_trace,
    )
    outs = [np.asarray(res.results[i]["y"]).astype(np.float32) for i in range(NCORES)]
    if _tokens == T:
        full = np.stack([o.reshape(V, P, L, D) for o in outs], axis=0)
    else:
        full = np.stack(outs, axis=0)
    if _trace:
        return full, res
    return full
